# revision 1
# baseline (speedup 1.0000x reference)
"""Self-contained Trainium2 Bass kernel for nn_AdExternal_N3Tree.

kernel(**inputs) takes the FULL unsharded inputs and returns the FULL
[210001, 4] output. Internally: host-side tree parsing/sharding prep,
then two SPMD launches on 8 NeuronCores:
  launch 1: per-parent conv-chain recurrence -> partial weighted feats
  launch 2: feats-shifted-bias 2-layer MLP over all 240000 cells
Host work is limited to index prep, sharding/marshalling, and the
unshard (sum of 8 partial feat vectors, row gather of leaf cells).
"""
"""N3Tree kernel: host prep + two-launch Bass implementation.

Launch 1 (conv): per-parent chain feat recurrence, sharded over groups,
outputs per-core partial weighted-feat sums (+ root final feat).
Launch 2 (MLP): feats-shifted bias, 2-layer MLP over all 240000 cells,
sharded over nodes.
"""
import numpy as np

N_CORES = 8
M_NODES = 30000
S = 8
D = 32
NODES_PER_CORE = M_NODES // N_CORES  # 3750
DEPTH_LIMIT = 10

# ---------------------------------------------------------------------------
# Host prep
# ---------------------------------------------------------------------------

def prep(inputs):
    """Parse tree structure, build all per-core arrays + emission metadata."""
    idx_sorted = np.asarray(inputs["idx_sorted"])
    depth_sorted = np.asarray(inputs["depth_sorted"])
    node_depth = np.asarray(inputs["node_depth"])
    depth_weight = np.asarray(inputs["depth_weight"])
    data = np.asarray(inputs["data"]).reshape(M_NODES, S * D)  # [node, v=k*32+i]
    conv_w = np.asarray(inputs["conv_w"])  # [10, o, i, k]
    conv_b = np.asarray(inputs["conv_b"])  # [10, 32]
    leaf_idx = np.asarray(inputs["leaf_idx"])

    n_steps = len(idx_sorted)
    wstep = depth_weight[depth_sorted].astype(np.float64)  # positional weights

    p_all = (idx_sorted // S).astype(np.int64)
    c_all = (idx_sorted % S).astype(np.int64)

    # fold duplicate packs (artifact): step i with idx == idx[i-1] merges into i-1
    dup = np.zeros(n_steps, bool)
    dup[1:] = idx_sorted[1:] == idx_sorted[:-1]
    # accumulate weights backward onto the first of each run of equal packs
    w_eff = wstep.copy()
    # runs of equal packs are length <= 2 here, but handle general case
    for i in range(n_steps - 1, 0, -1):
        if dup[i]:
            w_eff[i - 1] += w_eff[i]
    keep = ~dup
    p_k, c_k, w_k = p_all[keep], c_all[keep], w_eff[keep]

    # groups: runs of equal p (p_k descending)
    change = np.nonzero(np.diff(p_k))[0] + 1
    starts = np.concatenate([[0], change])
    ends = np.concatenate([change, [len(p_k)]])
    parents = p_k[starts]
    sizes = (ends - starts).astype(np.int64)
    depths = node_depth[parents].astype(np.int64)
    n_groups = len(parents)
    max_size = int(sizes.max())

    # per-group cells / weights arrays padded to max_size
    cells = np.zeros((n_groups, max_size), np.int64)
    ws = np.zeros((n_groups, max_size), np.float64)
    for g, (s0, e0) in enumerate(zip(starts, ends)):
        cells[g, : e0 - s0] = c_k[s0:e0]
        ws[g, : e0 - s0] = w_k[s0:e0]

    # ---- global sort: (size desc, depth asc), pad each (size, depth) run to %8
    order = np.lexsort((depths, -sizes))
    parents, sizes, depths = parents[order], sizes[order], depths[order]
    cells, ws = cells[order], ws[order]

    # build padded global list
    gp, gs, gd, gc, gw, is_dummy = [], [], [], [], [], []
    i = 0
    runs = []  # (size, depth, padded_len) in order
    while i < n_groups:
        s_val, d_val = sizes[i], depths[i]
        j = i
        while j < n_groups and sizes[j] == s_val and depths[j] == d_val:
            j += 1
        run_len = j - i
        pad = (-run_len) % N_CORES
        for t in range(i, j):
            gp.append(parents[t]); gs.append(s_val); gd.append(d_val)
            gc.append(cells[t]); gw.append(ws[t]); is_dummy.append(False)
        for _ in range(pad):
            gp.append(-1); gs.append(s_val); gd.append(d_val)
            gc.append(np.zeros(max_size, np.int64))
            gw.append(np.zeros(max_size)); is_dummy.append(False or True)
        runs.append((int(s_val), int(d_val), run_len + pad))
        i = j
    gp = np.array(gp); gs = np.array(gs); gd = np.array(gd)
    gc = np.array(gc); gw = np.array(gw); is_dummy = np.array(is_dummy)
    n_pad = len(gp)
    assert n_pad % N_CORES == 0
    G = n_pad // N_CORES  # per-core group count

    # per-core deal: core c gets global positions c, c+8, ...
    # per-core column j <-> global position j*8 + c
    # run boundaries in per-core space: cumulative(run_len/8)
    col_runs = []  # (size, depth, start_col, end_col) in per-core space
    acc = 0
    for s_val, d_val, L in runs:
        col_runs.append((s_val, d_val, acc, acc + L // N_CORES))
        acc += L // N_CORES
    assert acc == G

    # per-round active count (same for all cores): groups with size > r
    # column order is size-desc so active set is prefix [0, A_r)
    A = []  # A[r] for r = 1..max_size-1 (update rounds)
    for r in range(1, max_size):
        A.append(int((gs > r).sum() // N_CORES))

    # ---- per-core arrays
    # parent blocks transposed: par[core][v, g] = data[parent, v]
    par = np.zeros((N_CORES, 256, G), np.float32)
    for c in range(N_CORES):
        sel = gp[c::N_CORES]
        valid = sel >= 0
        par[c][:, valid] = data[sel[valid]].T

    # masks / negx0 / wexp concatenated over rounds
    sumA = int(sum(A))
    maskexp = np.zeros((N_CORES, 256, sumA), np.float32)
    negx0 = np.zeros((N_CORES, 256, sumA), np.float32)
    wexp0 = np.zeros((N_CORES, 32, G), np.float32)
    wexpR = np.zeros((N_CORES, 32, sumA), np.float32)
    offs = np.concatenate([[0], np.cumsum(A)]).astype(int)  # offsets per round
    for c in range(N_CORES):
        cg = gc[c::N_CORES]   # [G, max_size]
        wg = gw[c::N_CORES]
        wexp0[c][:, :] = wg[:, 0][None, :]
        for r in range(1, max_size):
            a = A[r - 1]
            off = offs[r - 1]
            # round r uses cell c_{r-1} (the previously-written cell)
            cc = cg[:a, r - 1]
            rows = (cc[None, :] * 32 + np.arange(32)[:, None])  # [32, a]
            colj = np.broadcast_to(np.arange(a)[None, :], rows.shape)
            maskexp[c][rows, off + colj] = 1.0
            negx0[c][rows, off + colj] = -par[c][rows, colj]
            wexpR[c][:, off : off + a] = wg[:a, r][None, :]

    # weights: Wtrep [10, 2, 128, 128]; lhsT[v', 32*a+o] = W[d, o, i, k],
    # v = 128*half + v' = k*32 + i
    Wtrep = np.zeros((DEPTH_LIMIT, 2, 128, 128), np.float32)
    wt = conv_w.transpose(0, 3, 2, 1).reshape(DEPTH_LIMIT, 256, 32)  # [d, v, o]
    for a in range(4):
        Wtrep[:, 0, :, 32 * a : 32 * a + 32] = wt[:, :128, :]
        Wtrep[:, 1, :, 32 * a : 32 * a + 32] = wt[:, 128:, :]
    stackI = np.zeros((32, 128), np.float32)
    for a in range(4):
        stackI[:, 32 * a : 32 * a + 32] = np.eye(32, dtype=np.float32)
    WtrepI = Wtrep + np.tile(np.eye(32, dtype=np.float32), (4, 4)).reshape(1, 1, 128, 128)
    # x0rep: +x0 values replicated to all four 32-blocks [core, 128, sumA]
    x0rep = np.zeros((N_CORES, 128, sumA), np.float32)
    for c in range(N_CORES):
        x0vals = -(negx0[c][:128].reshape(4, 32, sumA).sum(0)
                   + negx0[c][128:].reshape(4, 32, sumA).sum(0))
        x0rep[c] = np.tile(x0vals, (4, 1))
    # conv bias replicated: brep[d, 32*a+o] = conv_b[d, o]
    brep = np.tile(conv_b, (1, 4)).astype(np.float32)  # [10, 128]
    has_conv_b = bool(np.any(conv_b != 0))

    # root-patch info
    root_pos = int(np.nonzero(gp == 0)[0][0])
    root_core, root_col = root_pos % N_CORES, root_pos // N_CORES
    cell0_is_leaf = bool(leaf_idx[0] == 0)

    # concatenated DMA buffers
    # wtall [128, (set,d,h,m)]: set 0 = Wtrep, set 1 = WtrepI
    wtall = np.zeros((128, 2 * DEPTH_LIMIT * 2 * 128), np.float32)
    for st, Wsrc in enumerate((Wtrep, WtrepI)):
        for d in range(DEPTH_LIMIT):
            for h in range(2):
                col = ((st * DEPTH_LIMIT + d) * 2 + h) * 128
                wtall[:, col : col + 128] = Wsrc[d, h]
    # roundbuf [core, 128, 5*sumA]: per round r: [mlo|mhi|nxlo|nxhi|x0rep]
    roundbuf = np.zeros((N_CORES, 128, 5 * max(sumA, 1)), np.float32)
    for c in range(N_CORES):
        for r in range(1, max_size):
            a = A[r - 1]; off = offs[r - 1]; base = 5 * off
            roundbuf[c][:, base : base + a] = maskexp[c][:128, off : off + a]
            roundbuf[c][:, base + a : base + 2 * a] = maskexp[c][128:, off : off + a]
            roundbuf[c][:, base + 2 * a : base + 3 * a] = negx0[c][:128, off : off + a]
            roundbuf[c][:, base + 3 * a : base + 4 * a] = negx0[c][128:, off : off + a]
            roundbuf[c][:, base + 4 * a : base + 5 * a] = x0rep[c][:, off : off + a]
    # wexpall [core, 32, G + sumA]
    wexpall = np.concatenate([wexp0, wexpR], axis=2)

    meta = dict(
        G=G, A=A, offs=offs, col_runs=col_runs, max_size=max_size,
        has_conv_b=has_conv_b, root_core=root_core, root_col=root_col,
        cell0_is_leaf=cell0_is_leaf, sumA=sumA,
    )
    arrays = dict(par=par, maskexp=maskexp, negx0=negx0, wexp0=wexp0,
                  wexpR=wexpR, Wtrep=Wtrep, WtrepI=WtrepI, x0rep=x0rep,
                  stackI=stackI, brep=brep, wtall=wtall, roundbuf=roundbuf,
                  wexpall=wexpall)
    return meta, arrays



"""Bass builders for the two N3Tree launches (fp16 data path)."""
import sys
sys.path.insert(0, "/opt/trn_rl_repo")
import numpy as np
import concourse.bass as bass
import concourse.tile as tile
from concourse import bacc, mybir

F32 = mybir.dt.float32
F16 = mybir.dt.float16
MULT = mybir.AluOpType.mult
ADD = mybir.AluOpType.add
N_CORES = 8
NODES = 3750      # real nodes per core
NODES_DEV = 4096  # padded to 8 chunks of 512 (bank-aligned slices)
S, D = 8, 32
GELU = mybir.ActivationFunctionType.Gelu
DEPTH_LIMIT = 10


def ceil_div(a, b):
    return (a + b - 1) // b


# ---------------------------------------------------------------------------
# Launch 2: MLP over all cells of this core's node range
# ---------------------------------------------------------------------------

def build_launch2(has_b1=False, has_b2=False, chunk=512, act_func=None,
                  nodes_dev=NODES_DEV, psum_init=False):
    """MLP over all cells. fp16 data path, f32 accumulation.

    For each (chunk ci, k-quad q): 4 slices (k=4q..4q+3). Layer-1: row-tiled
    fp16 matmuls, two hp psum tiles of 2 slices (distinct banks). gelu per hp
    tile -> hs fp16. Layer-2: 4 col-tiled matmuls into one p2 bank at
    partition slices 32j. One dense copy -> rotating persistent stage tile,
    one DMA per quad into out_dev[quad]; host unpacks rows."""
    act_func = act_func or GELU
    nc = bacc.Bacc(None, target_bir_lowering=False)
    n_chunks = nodes_dev // chunk
    n_quads = n_chunks * 2
    dt = nc.dram_tensor("dt", [128, n_chunks, 2 * chunk], F16, kind="ExternalInput")
    w1 = nc.dram_tensor("w1", [32, 128], F32, kind="ExternalInput")
    w1rep = nc.dram_tensor("w1rep", [128, 128], F16, kind="ExternalInput")
    b1 = nc.dram_tensor("b1", [1, 128], F32, kind="ExternalInput")
    w2 = nc.dram_tensor("w2", [128, 4], F16, kind="ExternalInput")
    b2 = nc.dram_tensor("b2", [1, 4], F32, kind="ExternalInput")
    feats = nc.dram_tensor("feats", [32, 1], F32, kind="ExternalInput")
    out = nc.dram_tensor("out", [n_quads, 100, chunk], F32, kind="ExternalOutput")

    with tile.TileContext(nc) as tc:
        with (
            tc.tile_pool(name="const", bufs=1) as constp,
            tc.tile_pool(name="dtp", bufs=4) as dtp,
            tc.tile_pool(name="hps", bufs=3, space=bass.MemorySpace.PSUM) as hps,
            tc.tile_pool(name="ps2", bufs=2, space=bass.MemorySpace.PSUM) as ps2,
            tc.tile_pool(name="hsb", bufs=4) as hsb,
            tc.tile_pool(name="stg", bufs=1) as stgp,
        ):
            w1t = constp.tile([32, 128], F32, tag="w1t")
            nc.scalar.dma_start(w1t[:], w1[:])
            w1rept = constp.tile([128, 128], F16, tag="w1rept")
            nc.scalar.dma_start(w1rept[:], w1rep[:])
            w2t = constp.tile([128, 4], F16, tag="w2t")
            nc.scalar.dma_start(w2t[:], w2[:])
            featst = constp.tile([32, 1], F32, tag="featst")
            nc.scalar.dma_start(featst[:], feats[:])
            ones = constp.tile([1, 512], F32, tag="ones")
            nc.gpsimd.memset(ones[:], 1.0)
            zrow = constp.tile([1, 128], F32, tag="zrow")
            nc.gpsimd.memset(zrow[:], 0.0)

            # bias128 = w1.T @ feats (+ b1)   (f32 path)
            biasps = ps2.tile([128, 512], F32, tag="ps2", name="biasps")
            nc.tensor.matmul(biasps[:, 0:1], w1t[:], featst[:],
                             start=True, stop=not has_b1)
            if has_b1:
                b1t = constp.tile([1, 128], F32, tag="b1t")
                nc.scalar.dma_start(b1t[:], b1[:])
                nc.tensor.matmul(biasps[:, 0:1], b1t[:], ones[:, 0:1],
                                 start=False, stop=True)
            bias128 = constp.tile([128, 1], F32, tag="bias128")
            nc.vector.tensor_copy(bias128[:], biasps[:, 0:1])
            if has_b2:
                b2t = constp.tile([1, 4], F32, tag="b2t")
                nc.scalar.dma_start(b2t[:], b2[:])

            # persistent stage tiles (memset once so DMA reads are defined)
            stages = []
            for si in range(3):
                st = stgp.tile([128, chunk], F32, tag=f"stage{si}",
                               name=f"stage{si}")
                nc.gpsimd.memset(st[:], 0.0)
                stages.append(st)

            dt_tiles = {}

            def get_dt(ci):
                if ci not in dt_tiles:
                    t = dtp.tile([128, 2 * chunk], F16, tag="dt", name=f"dt{ci}")
                    nc.sync.dma_start(t[:], dt[:, ci, :])
                    dt_tiles[ci] = t
                return dt_tiles[ci]

            qi = 0
            for ci in range(n_chunks):
                for q in range(2):
                    hs_list = []
                    for sub in range(2):
                        hp = hps.tile([128, 2 * chunk], F32, tag="hps",
                                      name=f"hp{qi}_{sub}")
                        for jj in range(2):
                            k = 4 * q + 2 * sub + jj
                            half, kk = k // 4, k % 4
                            dtt = get_dt(ci)
                            nc.tensor.matmul(
                                hp[:, jj * chunk : (jj + 1) * chunk],
                                w1rept[32 * kk : 32 * kk + 32, :],
                                dtt[32 * kk : 32 * kk + 32,
                                    half * chunk : (half + 1) * chunk],
                                start=True, stop=True,
                                tile_position=(32 * kk, 0),
                            )
                        hs = hsb.tile([128, 2 * chunk], F16, tag="hsb",
                                      name=f"hs{qi}_{sub}")
                        nc.scalar.activation(hs[:], hp[:], act_func,
                                             bias=bias128[:], scale=1.0)
                        hs_list.append(hs)
                    p2 = ps2.tile([128, 512], F32, tag="ps2", name=f"p2_{qi}")
                    if psum_init:
                        nc.tensor.matmul(p2[:, :chunk], zrow[:], ones[:, :chunk],
                                         start=True, stop=True)
                    for j in range(4):
                        hs = hs_list[j // 2]
                        col0 = (j % 2) * chunk
                        nc.tensor.matmul(
                            p2[32 * j : 32 * j + 4, :chunk],
                            w2t[:, :],
                            hs[:, col0 : col0 + chunk],
                            start=True, stop=not has_b2,
                            tile_position=(0, 32 * j),
                        )
                        if has_b2:
                            nc.tensor.matmul(
                                p2[32 * j : 32 * j + 4, :chunk], b2t[:],
                                ones[:, :chunk],
                                start=False, stop=True, tile_position=(0, 32 * j),
                            )
                    st = stages[qi % 3]
                    nc.vector.tensor_copy(st[0:100, :chunk], p2[0:100, :chunk])
                    nc.sync.dma_start(out[qi], st[0:100, :chunk])
                    qi += 1
    nc.compile()
    return nc


# ---------------------------------------------------------------------------
# Launch 1: conv phase (fp16)
# ---------------------------------------------------------------------------

def build_launch1(meta, has_conv_b=False):
    G = meta["G"]
    A = meta["A"]
    offs = meta["offs"]
    col_runs = meta["col_runs"]
    sumA = meta["sumA"]
    n_rounds = len(A)
    root_col = meta["root_col"]

    nc = bacc.Bacc(None, target_bir_lowering=False)
    par = nc.dram_tensor("par", [2, 128, G], F16, kind="ExternalInput")
    roundbufd = nc.dram_tensor("roundbufd", [128, 5 * max(sumA, 1)], F16,
                               kind="ExternalInput")
    wexpalld = nc.dram_tensor("wexpalld", [32, G + sumA], F16,
                              kind="ExternalInput")
    wtalld = nc.dram_tensor("wtalld", [128, 2 * DEPTH_LIMIT * 2 * 128], F16,
                            kind="ExternalInput")
    brepd = nc.dram_tensor("brepd", [10, 128], F32, kind="ExternalInput")
    outs = nc.dram_tensor("outs", [32, n_rounds + 2], F32, kind="ExternalOutput")

    def bank_splits(c0, c1):
        res = []
        while c0 < c1:
            nxt = min(c1, (c0 // 512 + 1) * 512)
            res.append((c0, nxt))
            c0 = nxt
        return res

    with tile.TileContext(nc) as tc:
        with (
            tc.tile_pool(name="const", bufs=1) as constp,
            tc.tile_pool(name="feat", bufs=1) as featp,
            tc.tile_pool(name="rb", bufs=1) as rbp,
            tc.tile_pool(name="exp", bufs=2) as expp,
            tc.tile_pool(name="ps", bufs=8, space=bass.MemorySpace.PSUM) as psp,
            tc.tile_pool(name="scr", bufs=1) as scrp,
        ):
            wtall = constp.tile([128, 2 * DEPTH_LIMIT * 2 * 128], F16,
                                tag="wtall")
            nc.scalar.dma_start(wtall[:], wtalld[:])

            def wt_ap(st, d, h):
                col = ((st * DEPTH_LIMIT + d) * 2 + h) * 128
                return wtall[:, col : col + 128]

            ones = constp.tile([1, 512], F32, tag="ones")
            nc.gpsimd.memset(ones[:], 1.0)
            if has_conv_b:
                brept = constp.tile([10, 128], F32, tag="brept")
                nc.scalar.dma_start(brept[:], brepd[:])

            parlo = constp.tile([128, G], F16, tag="parlo")
            parhi = constp.tile([128, G], F16, tag="parhi")
            nc.sync.dma_start(parlo[:], par[0])
            nc.sync.dma_start(parhi[:], par[1])

            wexpall = constp.tile([32, G + sumA], F16, tag="wexpall")
            nc.scalar.dma_start(wexpall[:], wexpalld[:])

            feat128 = featp.tile([128, G], F16, tag="feat128")
            acc = constp.tile([32, n_rounds + 2], F32, tag="acc")

            # prefetch all round buffers up front (independent of compute)
            rbt = []
            for r in range(1, n_rounds + 1):
                a = A[r - 1]
                base = 5 * int(offs[r - 1])
                t = rbp.tile([128, 5 * a], F16, tag=f"rb{r}", name=f"rb{r}")
                nc.sync.dma_start(t[:], roundbufd[:, base : base + 5 * a])
                rbt.append(t)

            # ---- g1 ----
            n_banks = ceil_div(G, 512)
            g1ps = [psp.tile([128, 512], F32, tag="ps", name=f"g1ps{_i}")
                    for _i in range(n_banks)]
            for (s_val, d_val, c0, c1) in col_runs:
                for (b0, b1_) in bank_splits(c0, c1):
                    bk, o0 = b0 // 512, b0 % 512
                    o1 = o0 + (b1_ - b0)
                    nc.tensor.matmul(g1ps[bk][:, o0:o1], wt_ap(0, d_val, 0),
                                     parlo[:, b0:b1_], start=True, stop=False)
                    nc.tensor.matmul(g1ps[bk][:, o0:o1], wt_ap(0, d_val, 1),
                                     parhi[:, b0:b1_], start=False,
                                     stop=not has_conv_b)
                    if has_conv_b:
                        nc.tensor.matmul(g1ps[bk][:, o0:o1],
                                         brept[d_val : d_val + 1, :],
                                         ones[:, : b1_ - b0],
                                         start=False, stop=True)
            for bk in range(n_banks):
                w = min(512, G - bk * 512)
                nc.vector.tensor_copy(feat128[:, bk * 512 : bk * 512 + w],
                                      g1ps[bk][:, :w])
            scr = scrp.tile([32, max(G, 512)], F16, tag="scr")
            nc.vector.scalar_tensor_tensor(
                out=scr[:, :G], in0=feat128[0:32, :G], scalar=1.0,
                in1=wexpall[:, :G], op0=MULT, op1=MULT,
                accum_out=acc[:, 0:1])

            # ---- rounds ----
            for r in range(1, n_rounds + 1):
                a = A[r - 1]
                rb = rbt[r - 1]
                mlo, mhi = rb[:, 0:a], rb[:, a : 2 * a]
                nxlo, nxhi = rb[:, 2 * a : 3 * a], rb[:, 3 * a : 4 * a]
                x0rep = rb[:, 4 * a : 5 * a]
                explo = expp.tile([128, a], F16, tag="explo", bufs=1,
                                  name=f"explo{r}")
                exphi = expp.tile([128, a], F16, tag="exphi", bufs=1,
                                  name=f"exphi{r}")
                nc.vector.tensor_tensor(explo[:], feat128[:, :a], mlo, MULT)
                nc.vector.tensor_tensor(explo[:], explo[:], nxlo, ADD)
                nc.vector.tensor_tensor(exphi[:], feat128[:, :a], mhi, MULT)
                nc.vector.tensor_tensor(exphi[:], exphi[:], nxhi, ADD)
                updps = [psp.tile([128, 512], F32, tag="ps", name=f"updps{r}_{_i}")
                         for _i in range(ceil_div(a, 512))]
                for (s_val, d_val, c0, c1) in col_runs:
                    if s_val <= r or c0 >= a:
                        continue
                    c1 = min(c1, a)
                    for (b0, b1_) in bank_splits(c0, c1):
                        bk, o0 = b0 // 512, b0 % 512
                        o1 = o0 + (b1_ - b0)
                        ps = updps[bk]
                        nc.tensor.matmul(ps[:, o0:o1], wt_ap(1, d_val, 0),
                                         explo[:, b0:b1_], start=True, stop=False)
                        nc.tensor.matmul(ps[:, o0:o1], wt_ap(1, d_val, 1),
                                         exphi[:, b0:b1_], start=False, stop=True)
                for bk in range(ceil_div(a, 512)):
                    w = min(512, a - bk * 512)
                    nc.vector.tensor_tensor(
                        feat128[:, bk * 512 : bk * 512 + w],
                        updps[bk][:, :w],
                        x0rep[:, bk * 512 : bk * 512 + w], ADD)
                nc.vector.scalar_tensor_tensor(
                    out=scr[:, :a], in0=feat128[0:32, :a], scalar=1.0,
                    in1=wexpall[:, G + int(offs[r - 1]) : G + int(offs[r - 1]) + a],
                    op0=MULT, op1=MULT,
                    accum_out=acc[:, r : r + 1])

            nc.vector.tensor_copy(acc[:, n_rounds + 1 : n_rounds + 2],
                                  feat128[0:32, root_col : root_col + 1])
            nc.sync.dma_start(outs[:], acc[:])
    nc.compile()
    return nc

# ---------------------------------------------------------------------------
# Top-level kernel(): full inputs -> full output, two SPMD launches
# ---------------------------------------------------------------------------

_F16 = np.float16
_cache = {}
TRACE = False
LAST_EXEC_NS = {}


def _meta_key(meta, flags):
    return (meta["G"], meta["sumA"], tuple(meta["A"]), tuple(meta["col_runs"]),
            meta["root_col"], flags)


def kernel(**inputs):
    from concourse.bass_utils import run_bass_kernel_spmd
    inputs = {k: np.asarray(v) for k, v in inputs.items()}
    meta, arrays = prep(inputs)
    n_rounds = len(meta["A"])

    # ---- launch 1: conv phase ----
    k1 = ("l1",) + _meta_key(meta, (meta["has_conv_b"],))
    if k1 not in _cache:
        _cache[k1] = build_launch1(meta, has_conv_b=meta["has_conv_b"])
    nc1 = _cache[k1]
    wtall16 = np.ascontiguousarray(arrays["wtall"].astype(_F16))
    in1 = []
    for c in range(N_CORES):
        in1.append(dict(
            par=np.ascontiguousarray(
                arrays["par"][c].reshape(2, 128, meta["G"]).astype(_F16)),
            roundbufd=np.ascontiguousarray(arrays["roundbuf"][c].astype(_F16)),
            wexpalld=np.ascontiguousarray(arrays["wexpall"][c].astype(_F16)),
            wtalld=wtall16,
            brepd=np.ascontiguousarray(arrays["brep"]),
        ))
    res1 = run_bass_kernel_spmd(nc1, in1, core_ids=list(range(N_CORES)),
                                trace=TRACE)
    LAST_EXEC_NS["launch1"] = res1.exec_time_ns
    accs = np.stack([res1.results[c]["outs"] for c in range(N_CORES)])
    feats = accs[:, :, : n_rounds + 1].sum(axis=(0, 2)).astype(np.float32)
    rootfeat = accs[meta["root_core"], :, n_rounds + 1].astype(np.float32)

    # ---- launch 2: MLP over all cells ----
    data = inputs["data"].reshape(M_NODES, S * D).astype(np.float32)
    if meta["cell0_is_leaf"]:
        data = data.copy()
        data[0, :D] = rootfeat
    W1both = np.concatenate([inputs["hf_w1"], inputs["hs_w1"]], 1).astype(np.float32)
    b1both = np.concatenate([inputs["hf_b1"], inputs["hs_b1"]]).astype(np.float32)
    W2bd = np.zeros((128, 4), np.float32)
    W2bd[:64, :3] = inputs["hf_w2"]
    W2bd[64:, 3:] = inputs["hs_w2"]
    b2 = np.concatenate([inputs["hf_b2"], inputs["hs_b2"]]).astype(np.float32)
    has_b1 = bool(b1both.any())
    has_b2 = bool(b2.any())

    k2 = ("l2", has_b1, has_b2)
    if k2 not in _cache:
        _cache[k2] = build_launch2(has_b1=has_b1, has_b2=has_b2)
    nc2 = _cache[k2]

    w1rep = np.ascontiguousarray(np.tile(W1both, (4, 1)).astype(_F16))
    w2f16 = np.ascontiguousarray(W2bd.astype(_F16))
    n_chunks = NODES_DEV // 512
    in2 = []
    for c in range(N_CORES):
        blk = np.zeros((NODES_DEV, S * D), np.float32)
        blk[:NODES_PER_CORE] = data[c * NODES_PER_CORE : (c + 1) * NODES_PER_CORE]
        # [128, n_chunks, 1024]: [p, ci, h*512+f]
        dtc = blk.T.reshape(2, 128, n_chunks, 512).transpose(1, 2, 0, 3).reshape(
            128, n_chunks, 1024)
        in2.append(dict(
            dt=np.ascontiguousarray(dtc.astype(_F16)),
            w1=W1both, w1rep=w1rep, b1=b1both[None, :], w2=w2f16,
            b2=b2[None, :], feats=feats[:, None],
        ))
    res2 = run_bass_kernel_spmd(nc2, in2, core_ids=list(range(N_CORES)),
                                trace=TRACE)
    LAST_EXEC_NS["launch2"] = res2.exec_time_ns

    # ---- unshard: unpack [n_quads, 100, 512] mirrors -> [240000, 4] ----
    allout = np.stack([res2.results[c]["out"] for c in range(N_CORES)])
    # rows 32j..32j+4 of quad (ci,q) hold out4 for k=4q+j
    full = np.empty((N_CORES, 4, S, NODES_DEV), np.float32)
    for j in range(4):
        blkrows = allout[:, :, 32 * j : 32 * j + 4]  # [core, quad, 4, 512]
        q0 = blkrows.reshape(N_CORES, n_chunks, 2, 4, 512)
        for q in range(2):
            full[:, :, 4 * q + j] = q0[:, :, q].transpose(0, 2, 1, 3).reshape(
                N_CORES, 4, NODES_DEV)
    cells = full[:, :, :, :NODES_PER_CORE].transpose(0, 3, 2, 1).reshape(
        M_NODES * S, 4)
    return cells[inputs["leaf_idx"]]

